# revision 16
# baseline (speedup 1.0000x reference)
"""HypergraphConv + BatchNorm + SiLU on 8 Trainium2 NeuronCores.

out = SiLU(BN(D^-1 H B^-1 H^T (X W) + b))

Device strategy (all float math in bf16/f32-PSUM on device; host does
index-only work plus the exact B^-1/D^-1 reciprocals):
  - Aggregation is linear: aggregate raw x rows; apply W once per node
    tile at the end of phase B.
  - Segment-sum = dma_gather row gathers (256B bf16 rows) + one-hot
    selection built on DVE + TensorE matmul accumulation in PSUM.
  - Matmuls run in "transposed" orientation (out[feat, slots] =
    gathered.T @ onehot) with destination-sorted incidences so each
    chunk's one-hot only spans a narrow slot window -> the matmul's
    moving free dim is the window width (32) instead of 128.
  - Edges/nodes are owned by core (id % 8); destination rows within a
    tile are assigned in a scrambled order so incidence mass is uniform
    across rows (windows stay feasible).
  - B (edge degree) and D (weighted node degree) are index-only / cheap
    host reductions; their reciprocals ride in as small f32 tables.
  - b (bias) cancels exactly under training-mode BatchNorm -> dropped.
  - e-table AllGather across cores; BN stats (sum/sumsq via a Gram
    matmul) AllReduce; y stays resident in SBUF between phase B and the
    finalize pass; host un-permutes rows to natural node order.
"""

import numpy as np
import ml_dtypes

import concourse.bass as bass
import concourse.mybir as mybir
import concourse.tile as tile
from concourse import bacc
from concourse.bass_utils import run_bass_kernel_spmd

F32 = mybir.dt.float32
BF16 = mybir.dt.bfloat16
I16 = mybir.dt.int16
AF = mybir.ActivationFunctionType
OP = mybir.AluOpType
NPBF = ml_dtypes.bfloat16

P = 128
SP2 = 256         # supertile: 2 PSUM tiles of 128 slots share one gather
WIN = 32          # slot window width for windowed chunk matmuls
PAD_LT = 300.0    # one-hot "no match" sentinel (>= any window width)


class Dims:
    def __init__(self, N, E, NNZ, n_cores):
        self.N, self.E, self.NNZ, self.NC = N, E, NNZ, n_cores
        assert N % n_cores == 0 and E % n_cores == 0
        self.NS = N // n_cores
        self.ES = E // n_cores
        self.T1 = -(-self.ES // P)
        self.T2 = -(-self.NS // P)
        self.T1p = -(-self.T1 // 2)
        self.T2p = -(-self.T2 // 2)
        # gather-source shards (int16 indices => shard < 32768 rows)
        self.NSH1 = max(1, -(-N // 25000))
        self.SH1 = -(-N // self.NSH1)
        ER = n_cores * self.T1 * P          # e-table rows
        self.ER = ER
        self.NSH2 = max(1, -(-ER // 25088))
        self.SH2 = -(-ER // self.NSH2)
        self.C1s = None        # chunks per shard, side 1 (uniform/core)
        self.C2s = None
        self.W1 = None         # windows per shard: list of (off, wdt)
        self.W2 = None
        self.BN_EPS = 1e-5


def _windows(C, cap=SP2, w=WIN):
    """Chunk window schedule for a shard with C chunks."""
    if C <= 10:
        return [(0, cap)] * C
    stride = (cap - w) / (C - 1)
    return [(min(int(round(k * stride)), cap - w), w) for k in range(C)]


def _pack_group(dst, C, wins):
    """Greedy: chunk k takes incidences (sorted by dst) while dst <
    wins[k][0]+wins[k][1] and count < 128. Returns list of (start, end)
    per chunk or None if infeasible."""
    n = len(dst)
    out = []
    ptr = 0
    for k in range(C):
        o, w = wins[k]
        if ptr < n and dst[ptr] < o:
            return None
        e = ptr
        hi = o + w
        while e < n and e - ptr < P and dst[e] < hi:
            e += 1
        out.append((ptr, e))
        ptr = e
    if ptr != n:
        return None
    return out


def _wrap16(vals):
    """flat int array [n] (n % 128 == 0) -> [128, n//16] int16 in the
    dma_gather layout: flat i at partition i%16, column i//16, replicated
    8x across partition groups."""
    n = len(vals)
    a = np.zeros((16, n // 16), np.int16)
    a[np.arange(n) % 16, np.arange(n) // 16] = vals.astype(np.int16)
    return np.tile(a, (8, 1))


def _assign(core_deg, T):
    """Order a core's items by degree desc; item i -> tile i % T,
    row ((i // T) * 61 + 13) % 128 (scrambled so per-row mass is flat).
    Returns (tile, row) arrays indexed by local item id."""
    n = len(core_deg)
    order = np.argsort(-core_deg, kind="stable")
    tile_ = np.empty(n, np.int32)
    row_ = np.empty(n, np.int32)
    i = np.arange(n)
    tile_[order] = i % T
    row_[order] = ((i // T) * 61 + 13) % P
    return tile_, row_


def _side_tables(dst_core, dst_tile, dst_row, gsrc, T, n_shards, shard_sz,
                 src_limit, NC):
    """Group incidences by (core, tile, shard), sort by dst row, choose a
    uniform chunk count + window schedule per shard, emit idx/loc tables.

    Returns (Cs, wins, per_core list of (idx [T,P,Ct*8] i16,
    loc [T,P,Ct] bf16))."""
    shard = gsrc // shard_sz
    key = ((dst_core.astype(np.int64) * T + dst_tile) * n_shards + shard)
    order = np.lexsort((dst_row, key))
    k_s = key[order]
    d_s = dst_row[order]
    g_s = (gsrc - shard * shard_sz)[order]
    ngrp = NC * T * n_shards
    starts = np.searchsorted(k_s, np.arange(ngrp))
    ends = np.searchsorted(k_s, np.arange(ngrp) + 1)

    # per-shard uniform chunk count with feasibility under the schedule
    Cs = np.zeros(n_shards, np.int64)
    wins_all = []
    packs = {}
    for s in range(n_shards):
        gids = np.arange(ngrp)[np.arange(ngrp) % n_shards == s]
        cnts = ends[gids] - starts[gids]
        C = max(1, int(-(-cnts.max() // P))) if len(cnts) else 1
        base = C
        while True:
            wins = _windows(C)
            ok = True
            pk = {}
            for g in gids:
                dst = d_s[starts[g]:ends[g]]
                r = _pack_group(dst, C, wins)
                if r is None:
                    ok = False
                    break
                pk[g] = r
            if ok:
                break
            C += 1
            if C > base + 6:
                C = base
                wins = [(0, P)] * C
                for g in gids:
                    dst = d_s[starts[g]:ends[g]]
                    pk[g] = _pack_group(dst, C, wins)
                    assert pk[g] is not None
                break
        Cs[s] = C
        wins_all.append(wins)
        packs.update(pk)

    Ct = int(Cs.sum())
    offs = np.concatenate([[0], np.cumsum(Cs)]).astype(int)
    per_core = []
    for c in range(NC):
        idx = np.zeros((T, P, Ct * 8), np.int16)
        loc = np.full((T, P, Ct), PAD_LT, NPBF)
        for t in range(T):
            for s in range(n_shards):
                g = (c * T + t) * n_shards + s
                C = int(Cs[s])
                wins = wins_all[s]
                gv = np.zeros(C * P, np.int64)
                lv = np.full(C * P, PAD_LT, np.float32)
                for k, (a, b) in enumerate(packs[g]):
                    m = b - a
                    sl = slice(k * P, k * P + m)
                    gv[sl] = g_s[starts[g] + a:starts[g] + b]
                    lv[sl] = (d_s[starts[g] + a:starts[g] + b]
                              - wins[k][0])
                idx[t, :, offs[s] * 8:offs[s + 1] * 8] = _wrap16(gv)
                loc[t, :, offs[s]:offs[s + 1]] = (
                    lv.reshape(C, P).T.astype(NPBF))
        per_core.append((idx, loc))
    return Cs, wins_all, per_core


def preprocess(x, hyperedge_index, hyperedge_weight, d):
    ni = np.asarray(hyperedge_index[0]).astype(np.int64)
    ei = np.asarray(hyperedge_index[1]).astype(np.int64)
    w = np.asarray(hyperedge_weight, np.float64)
    NC = d.NC

    edeg = np.bincount(ei, minlength=d.E)
    Dw = np.bincount(ni, weights=w[ei], minlength=d.N)

    e_tile = np.empty(d.E, np.int32)
    e_row = np.empty(d.E, np.int32)
    n_tile = np.empty(d.N, np.int32)
    n_row = np.empty(d.N, np.int32)
    for c in range(NC):
        ge = np.arange(d.ES) * NC + c
        e_tile[ge], e_row[ge] = _assign(edeg[ge], d.T1)
        gn = np.arange(d.NS) * NC + c
        n_tile[gn], n_row[gn] = _assign(
            np.bincount(ni, minlength=d.N)[gn], d.T2)

    e_core = ei % NC
    n_core = ni % NC
    e_grow = (e_core * (d.T1 * P) + e_tile[ei] * P + e_row[ei])
    # e_grow indexed per incidence; also per edge for loc of edges:
    e_growe = (np.arange(d.E) % NC) * (d.T1 * P) + e_tile * P + e_row

    et_i = e_tile[ei]
    nt_i = n_tile[ni]
    d.C1s, d.W1, side1 = _side_tables(
        e_core, et_i // 2, e_row[ei] + P * (et_i % 2), ni,
        d.T1p, d.NSH1, d.SH1, d.N, NC)
    d.C2s, d.W2, side2 = _side_tables(
        n_core, nt_i // 2, n_row[ni] + P * (nt_i % 2), e_growe[ei],
        d.T2p, d.NSH2, d.SH2, d.ER, NC)

    binv_g = np.where(edeg > 0, 1.0 / np.maximum(edeg, 1), 0.0)
    dinv_g = np.where(Dw > 0, 1.0 / np.where(Dw > 0, Dw, 1.0), 0.0)

    per_core = []
    for c in range(NC):
        ge = np.arange(d.ES) * NC + c
        bv = np.zeros((P, d.T1), np.float32)
        bv[e_row[ge], e_tile[ge]] = binv_g[ge]
        gn = np.arange(d.NS) * NC + c
        dv = np.zeros((P, d.T2), np.float32)
        dv[n_row[gn], n_tile[gn]] = dinv_g[gn]
        perm = (n_tile[gn] * P + n_row[gn]).astype(np.int64)
        per_core.append(dict(
            g1i=side1[c][0], g1l=side1[c][1],
            g2i=side2[c][0], g2l=side2[c][1],
            binv=bv, dinv=dv,
            _perm=perm,
        ))
    return per_core


def ap3(t_ap, dims_):
    return bass.AP(t_ap.tensor, t_ap.offset, dims_)


def build(d):
    nc = bacc.Bacc("TRN2", target_bir_lowering=False, num_devices=d.NC,
                   num_swdge_queues=4)
    C1t = int(sum(d.C1s))
    C2t = int(sum(d.C2s))
    o1 = np.concatenate([[0], np.cumsum(d.C1s)]).astype(int)
    o2 = np.concatenate([[0], np.cumsum(d.C2s)]).astype(int)
    # one-hot column offsets per shard (widths can differ per shard)
    w1 = [d.W1[s][0][1] for s in range(d.NSH1)]
    w2 = [d.W2[s][0][1] for s in range(d.NSH2)]
    ow1 = np.concatenate([[0], np.cumsum([d.C1s[s] * w1[s]
                                          for s in range(d.NSH1)])]).astype(int)
    ow2 = np.concatenate([[0], np.cumsum([d.C2s[s] * w2[s]
                                          for s in range(d.NSH2)])]).astype(int)
    OHW1 = int(ow1[-1])
    OHW2 = int(ow2[-1])
    ER = d.ER
    YC = P + 1

    x_d = nc.dram_tensor("x", [d.N, P], BF16, kind="ExternalInput")
    w_d = nc.dram_tensor("Wm", [P, P], BF16, kind="ExternalInput")
    gm_d = nc.dram_tensor("gamma", [P, 1], F32, kind="ExternalInput")
    bt_d = nc.dram_tensor("beta", [P, 1], F32, kind="ExternalInput")
    g1i_d = nc.dram_tensor("g1i", [d.T1p, P, C1t * 8], I16, kind="ExternalInput")
    g1l_d = nc.dram_tensor("g1l", [d.T1p, P, C1t], BF16, kind="ExternalInput")
    g2i_d = nc.dram_tensor("g2i", [d.T2p, P, C2t * 8], I16, kind="ExternalInput")
    g2l_d = nc.dram_tensor("g2l", [d.T2p, P, C2t], BF16, kind="ExternalInput")
    bv_d = nc.dram_tensor("binv", [P, d.T1], F32, kind="ExternalInput")
    dv_d = nc.dram_tensor("dinv", [P, d.T2], F32, kind="ExternalInput")
    out_d = nc.dram_tensor("out", [d.T2 * P, P], BF16, kind="ExternalOutput")

    iotab_h = nc.inline_tensor(
        np.tile(np.arange(SP2, dtype=NPBF), (P, 1)), name="iota2db")
    ident_h = nc.inline_tensor(np.eye(P, dtype=np.float32), name="ident")

    groups = [list(range(d.NC))]

    with tile.TileContext(nc) as tc:
        with (
            tc.tile_pool(name="const", bufs=1) as cp,
            tc.tile_pool(name="dram", bufs=1, space="DRAM") as dp,
            tc.tile_pool(name="psS", bufs=1, space="PSUM") as psS,
            tc.tile_pool(name="ypool", bufs=1) as yp,
        ):
            IOTB = cp.tile([P, SP2], BF16, name="IOTB")
            nc.sync.dma_start(IOTB[:], iotab_h[:])
            IDN = cp.tile([P, P], F32, name="IDN")
            nc.sync.dma_start(IDN[:], ident_h[:])
            WSB = cp.tile([P, P], BF16, name="WSB")
            nc.sync.dma_start(WSB[:], w_d[:])
            GM = cp.tile([P, 1], F32, name="GM")
            nc.sync.dma_start(GM[:], gm_d[:])
            BT = cp.tile([P, 1], F32, name="BT")
            nc.sync.dma_start(BT[:], bt_d[:])
            BV = cp.tile([P, d.T1], F32, name="BV")
            nc.sync.dma_start(BV[:], bv_d[:])
            DV = cp.tile([P, d.T2], F32, name="DV")
            nc.sync.dma_start(DV[:], dv_d[:])
            ZRO = cp.tile([P, P], BF16, name="ZRO")
            nc.vector.memset(ZRO[:], 0.0)

            e_loc = dp.tile([d.T1 * P, P], BF16, name="e_loc")
            e_full = dp.tile([ER, P], BF16, name="e_full")
            st_in = dp.tile([P, 2], F32, name="st_in")
            st_out = dp.tile([P, 2], F32, name="st_out")

            stats_ps = psS.tile([P, P + 1], F32, name="stats_ps")
            Y = yp.tile([P, d.T2 * YC], BF16, name="Y")

            # ---------------- phase A: node -> edge ----------------
            with (
                tc.tile_pool(name="s1", bufs=3) as s1,
                tc.tile_pool(name="g1", bufs=2) as g1p,
                tc.tile_pool(name="ps1", bufs=2, space="PSUM") as ps1,
                tc.tile_pool(name="pst", bufs=2, space="PSUM") as pst,
            ):
                for tp in range(d.T1p):
                    it1 = s1.tile([P, C1t * 8], I16, name="it1")
                    nc.sync.dma_start(it1[:], g1i_d[tp])
                    lt1 = s1.tile([P, C1t], BF16, name="lt1")
                    nc.sync.dma_start(lt1[:], g1l_d[tp])
                    G1 = g1p.tile([P, C1t * P], BF16, name="G1")
                    for s in range(d.NSH1):
                        cs = int(d.C1s[s])
                        base = s * d.SH1
                        sz = min(d.SH1, d.N - base)
                        g_ap = G1[:, o1[s] * P:o1[s + 1] * P]
                        nc.gpsimd.dma_gather(
                            out_ap=ap3(g_ap, [g_ap.ap[0], [P, cs], [1, P]]),
                            in_ap=x_d[base:base + sz, :],
                            idxs_ap=it1[:, o1[s] * 8:o1[s + 1] * 8],
                            num_idxs=cs * P, num_idxs_reg=cs * P,
                            elem_size=P, single_packet=False,
                            queue_num=s % 4)
                    OH = g1p.tile([P, OHW1], BF16, name="OH")
                    for s in range(d.NSH1):
                        cs = int(d.C1s[s])
                        ws = w1[s]
                        oh_ap = OH[:, ow1[s]:ow1[s + 1]]
                        nc.vector.tensor_tensor(
                            out=ap3(oh_ap, [oh_ap.ap[0], [ws, cs], [1, ws]]),
                            in0=lt1[:, o1[s]:o1[s + 1]].to_broadcast(
                                [P, cs, ws]),
                            in1=ap3(IOTB[:], [IOTB[:].ap[0], [0, cs],
                                              [1, ws]]),
                            op=OP.is_equal)
                    peT = ps1.tile([P, SP2], F32, name="peT")
                    nc.tensor.matmul(peT[:], lhsT=ZRO[:], rhs=IOTB[:],
                                     start=True, stop=False,
                                     skip_group_check=True)
                    ci = 0
                    for s in range(d.NSH1):
                        cs = int(d.C1s[s])
                        ws = w1[s]
                        for k in range(cs):
                            ok, wk = d.W1[s][k]
                            nc.tensor.matmul(
                                peT[:, ok:ok + wk],
                                lhsT=G1[:, (o1[s] + k) * P:(o1[s] + k + 1) * P],
                                rhs=OH[:, ow1[s] + k * ws:ow1[s] + (k + 1) * ws],
                                start=False, stop=(ci == C1t - 1),
                                skip_group_check=True)
                            ci += 1
                    agt = s1.tile([P, SP2], F32, name="agt")
                    nc.vector.tensor_copy(agt[:], peT[:])
                    for j in range(2):
                        t = 2 * tp + j
                        if t >= d.T1:
                            break
                        pt = pst.tile([P, P], F32, name="pt")
                        nc.tensor.transpose(pt[:], agt[:, j * P:(j + 1) * P],
                                            IDN[:])
                        es = s1.tile([P, P], BF16, name="es")
                        nc.vector.tensor_scalar_mul(
                            out=es[:], in0=pt[:], scalar1=BV[:, t:t + 1])
                        nc.sync.dma_start(e_loc[t * P:(t + 1) * P, :], es[:])

            nc.gpsimd.collective_compute(
                "AllGather", OP.bypass, replica_groups=groups,
                ins=[e_loc[:]], outs=[e_full[:]])

            # ---------------- phase B: edge -> node ----------------
            with (
                tc.tile_pool(name="s2", bufs=3) as s2,
                tc.tile_pool(name="g2", bufs=2) as g2p,
                tc.tile_pool(name="ps2", bufs=2, space="PSUM") as ps2,
                tc.tile_pool(name="psz", bufs=2, space="PSUM") as psz,
            ):
                for tp in range(d.T2p):
                    it2 = s2.tile([P, C2t * 8], I16, name="it2")
                    nc.sync.dma_start(it2[:], g2i_d[tp])
                    lt2 = s2.tile([P, C2t], BF16, name="lt2")
                    nc.sync.dma_start(lt2[:], g2l_d[tp])
                    G2 = g2p.tile([P, C2t * P], BF16, name="G2")
                    for s in range(d.NSH2):
                        cs = int(d.C2s[s])
                        base = s * d.SH2
                        sz = min(d.SH2, ER - base)
                        g_ap = G2[:, o2[s] * P:o2[s + 1] * P]
                        nc.gpsimd.dma_gather(
                            out_ap=ap3(g_ap, [g_ap.ap[0], [P, cs], [1, P]]),
                            in_ap=e_full[base:base + sz, :],
                            idxs_ap=it2[:, o2[s] * 8:o2[s + 1] * 8],
                            num_idxs=cs * P, num_idxs_reg=cs * P,
                            elem_size=P, single_packet=False,
                            queue_num=(tp * d.NSH2 + s) % 4)
                    OH2 = g2p.tile([P, OHW2], BF16, name="OH2")
                    for s in range(d.NSH2):
                        cs = int(d.C2s[s])
                        ws = w2[s]
                        oh_ap = OH2[:, ow2[s]:ow2[s + 1]]
                        nc.vector.tensor_tensor(
                            out=ap3(oh_ap, [oh_ap.ap[0], [ws, cs], [1, ws]]),
                            in0=lt2[:, o2[s]:o2[s + 1]].to_broadcast(
                                [P, cs, ws]),
                            in1=ap3(IOTB[:], [IOTB[:].ap[0], [0, cs],
                                              [1, ws]]),
                            op=OP.is_equal)
                    snT = ps2.tile([P, SP2], F32, name="snT")
                    nc.tensor.matmul(snT[:], lhsT=ZRO[:], rhs=IOTB[:],
                                     start=True, stop=False,
                                     skip_group_check=True)
                    ci = 0
                    for s in range(d.NSH2):
                        cs = int(d.C2s[s])
                        ws = w2[s]
                        for k in range(cs):
                            ok, wk = d.W2[s][k]
                            nc.tensor.matmul(
                                snT[:, ok:ok + wk],
                                lhsT=G2[:, (o2[s] + k) * P:(o2[s] + k + 1) * P],
                                rhs=OH2[:, ow2[s] + k * ws:ow2[s] + (k + 1) * ws],
                                start=False, stop=(ci == C2t - 1),
                                skip_group_check=True)
                            ci += 1
                    agT = s2.tile([P, SP2], BF16, name="agT")
                    nc.vector.tensor_copy(agT[:], snT[:])
                    for j in range(2):
                        t = 2 * tp + j
                        if t >= d.T2:
                            break
                        z = psz.tile([P, P], F32, name="z")
                        nc.tensor.matmul(z[:], lhsT=agT[:, j * P:(j + 1) * P],
                                         rhs=WSB[:], start=True, stop=True)
                        yc = t * YC
                        nc.vector.tensor_scalar_mul(
                            out=Y[:, yc:yc + P], in0=z[:],
                            scalar1=DV[:, t:t + 1])
                        nc.vector.memset(Y[:, yc + P:yc + P + 1], 1.0)
                        nc.tensor.matmul(stats_ps[:], lhsT=Y[:, yc:yc + P],
                                         rhs=Y[:, yc:yc + P + 1],
                                         start=(t == 0), stop=(t == d.T2 - 1))

            # ---------------- phase C: BN stats ----------------
            with (
                tc.tile_pool(name="s3", bufs=1) as s3,
                tc.tile_pool(name="ps3", bufs=2, space="PSUM") as ps3,
            ):
                sts = s3.tile([P, P + 1], F32, name="sts")
                nc.vector.tensor_copy(sts[:], stats_ps[:])
                dg = s3.tile([P, P], F32, name="dg")
                nc.vector.tensor_tensor(out=dg[:], in0=sts[:, 0:P],
                                        in1=IDN[:], op=OP.mult)
                st2 = s3.tile([P, 2], F32, name="st2")
                nc.vector.tensor_reduce(out=st2[:, 1:2], in_=dg[:],
                                        axis=mybir.AxisListType.X, op=OP.add)
                nc.vector.tensor_copy(st2[:, 0:1], sts[:, P:P + 1])
                nc.sync.dma_start(st_in[:], st2[:])
                nc.gpsimd.collective_compute(
                    "AllReduce", OP.add, replica_groups=groups,
                    ins=[st_in[:]], outs=[st_out[:]])
                gst = s3.tile([P, 2], F32, name="gst")
                nc.sync.dma_start(gst[:], st_out[:])
                mean = s3.tile([P, 1], F32, name="mean")
                nc.vector.tensor_scalar_mul(out=mean[:], in0=gst[:, 0:1],
                                            scalar1=1.0 / d.N)
                var = s3.tile([P, 1], F32, name="var")
                nc.vector.tensor_scalar_mul(out=var[:], in0=gst[:, 1:2],
                                            scalar1=1.0 / d.N)
                m2 = s3.tile([P, 1], F32, name="m2")
                nc.vector.tensor_tensor(out=m2[:], in0=mean[:], in1=mean[:],
                                        op=OP.mult)
                nc.vector.tensor_tensor(out=var[:], in0=var[:], in1=m2[:],
                                        op=OP.subtract)
                epsl = s3.tile([P, 1], F32, name="epsl")
                nc.vector.memset(epsl[:], d.BN_EPS)
                sd = s3.tile([P, 1], F32, name="sd")
                nc.scalar.activation(out=sd[:], in_=var[:], func=AF.Sqrt,
                                     bias=epsl[:])
                nc.vector.reciprocal(sd[:], sd[:])
                scl = s3.tile([P, 1], F32, name="scl")
                nc.vector.tensor_tensor(out=scl[:], in0=GM[:], in1=sd[:],
                                        op=OP.mult)
                sft = s3.tile([P, 1], F32, name="sft")
                nc.vector.tensor_tensor(out=sft[:], in0=mean[:], in1=scl[:],
                                        op=OP.mult)
                nc.vector.tensor_tensor(out=sft[:], in0=BT[:], in1=sft[:],
                                        op=OP.subtract)
                pb = ps3.tile([P, P], F32, name="pb")
                nc.tensor.transpose(pb[:], scl[:].to_broadcast([P, P]),
                                    IDN[:])
                SCL = s3.tile([P, P], BF16, name="SCL")
                nc.vector.tensor_copy(SCL[:], pb[:])
                pb2 = ps3.tile([P, P], F32, name="pb2")
                nc.tensor.transpose(pb2[:], sft[:].to_broadcast([P, P]),
                                    IDN[:])
                SFT = s3.tile([P, P], BF16, name="SFT")
                nc.vector.tensor_copy(SFT[:], pb2[:])

                # ---------------- phase D: finalize ----------------
                with tc.tile_pool(name="s4", bufs=3) as s4:
                    for t in range(d.T2):
                        yc = t * YC
                        yt = s4.tile([P, P], BF16, name="yt")
                        nc.vector.tensor_tensor(out=yt[:],
                                                in0=Y[:, yc:yc + P],
                                                in1=SCL[:], op=OP.mult)
                        nc.vector.tensor_tensor(out=yt[:], in0=yt[:],
                                                in1=SFT[:], op=OP.add)
                        ot = s4.tile([P, P], BF16, name="ot")
                        nc.scalar.activation(out=ot[:], in_=yt[:],
                                             func=AF.Silu)
                        nc.sync.dma_start(out_d[t * P:(t + 1) * P, :], ot[:])
    nc.compile()
    return nc


def _run(d, x, W, gamma, beta, per_core, trace=False, **rkw):
    nc = build(d)
    xbf = np.ascontiguousarray(x.astype(NPBF))
    Wbf = np.ascontiguousarray(W.astype(NPBF))
    in_maps = []
    for c in range(d.NC):
        m = {k: v for k, v in per_core[c].items() if not k.startswith("_")}
        m["x"] = xbf
        m["Wm"] = Wbf
        m["gamma"] = gamma.reshape(P, 1)
        m["beta"] = beta.reshape(P, 1)
        in_maps.append(m)
    res = run_bass_kernel_spmd(nc, in_maps, core_ids=list(range(d.NC)),
                               trace=trace, **rkw)
    out = np.empty((d.N, P), np.float32)
    for c in range(d.NC):
        perm = per_core[c]["_perm"]
        oc = np.asarray(res.results[c]["out"]).astype(np.float32)
        out[c::d.NC] = oc[perm]
    return out, res


def kernel(x, hyperedge_index, hyperedge_weight, W, b, gamma, beta):
    x = np.ascontiguousarray(np.asarray(x, np.float32))
    W = np.ascontiguousarray(np.asarray(W, np.float32))
    gamma = np.asarray(gamma, np.float32)
    beta = np.asarray(beta, np.float32)
    d = Dims(N=x.shape[0], E=np.asarray(hyperedge_weight).shape[0],
             NNZ=np.asarray(hyperedge_index).shape[1], n_cores=8)
    per_core = preprocess(x, hyperedge_index, hyperedge_weight, d)
    out, _ = _run(d, x, W, gamma, beta, per_core)
    return out


# revision 27
# speedup vs baseline: 1.1075x; 1.1075x over previous
"""HypergraphConv + BatchNorm + SiLU on 8 Trainium2 NeuronCores.

out = SiLU(BN(D^-1 H B^-1 H^T (X W) + b))

Device strategy (all float math in bf16/f32-PSUM on device; host does
index-only work plus the exact B^-1/D^-1 reciprocals):
  - Aggregation is linear: aggregate raw x rows; apply W once per node
    tile at the end of phase B.
  - Segment-sum = dma_gather row gathers (256B bf16 rows) + one-hot
    selection built on DVE + TensorE matmul accumulation in PSUM.
  - Matmuls run in "transposed" orientation (out[feat, slots] =
    gathered.T @ onehot) with destination-sorted incidences so each
    chunk's one-hot only spans a narrow slot window -> the matmul's
    moving free dim is the window width (32) instead of 128.
  - Edges/nodes are owned by core (id % 8); destination rows within a
    tile are assigned in a scrambled order so incidence mass is uniform
    across rows (windows stay feasible).
  - B (edge degree) and D (weighted node degree) are index-only / cheap
    host reductions; their reciprocals ride in as small f32 tables.
  - b (bias) cancels exactly under training-mode BatchNorm -> dropped.
  - e-table AllGather across cores; BN stats (sum/sumsq via a Gram
    matmul) AllReduce; y stays resident in SBUF between phase B and the
    finalize pass; host un-permutes rows to natural node order.
"""

import numpy as np
import ml_dtypes

import concourse.bass as bass
import concourse.mybir as mybir
import concourse.tile as tile
from concourse import bacc
from concourse.bass_utils import run_bass_kernel_spmd

F32 = mybir.dt.float32
BF16 = mybir.dt.bfloat16
I16 = mybir.dt.int16
AF = mybir.ActivationFunctionType
OP = mybir.AluOpType
NPBF = ml_dtypes.bfloat16

P = 128
SP2 = 256         # supertile: 2 PSUM tiles of 128 slots share one gather
WIN = 32          # slot window width for windowed chunk matmuls
PAD_LT = 300.0    # one-hot "no match" sentinel (>= any window width)


class Dims:
    def __init__(self, N, E, NNZ, n_cores):
        self.N, self.E, self.NNZ, self.NC = N, E, NNZ, n_cores
        assert N % n_cores == 0 and E % n_cores == 0
        self.NS = N // n_cores
        self.ES = E // n_cores
        self.T1 = -(-self.ES // P)
        self.T2 = -(-self.NS // P)
        self.T1p = -(-self.T1 // 2)
        self.T2p = -(-self.T2 // 2)
        # gather-source shards (int16 indices => shard < 32768 rows)
        self.NSH1 = max(1, -(-N // 25000))
        self.SH1 = -(-N // self.NSH1)
        ER = n_cores * self.T1 * P          # e-table rows
        self.ER = ER
        self.NSH2 = max(1, -(-ER // 25088))
        self.SH2 = -(-ER // self.NSH2)
        self.C1s = None        # chunks per shard, side 1 (uniform/core)
        self.C2s = None
        self.W1 = None         # windows per shard: list of (off, wdt)
        self.W2 = None
        self.BN_EPS = 1e-5


def _windows(C, cap=P, w=WIN):
    """Chunk window schedule for a shard with C chunks."""
    if C <= 5:
        return [(0, cap)] * C
    stride = (cap - w) / (C - 1)
    return [(min(int(round(k * stride)), cap - w), w) for k in range(C)]


def _pack_group(dst, C, wins):
    """Greedy: chunk k takes incidences (sorted by dst) while dst <
    wins[k][0]+wins[k][1] and count < 128. Returns list of (start, end)
    per chunk or None if infeasible."""
    n = len(dst)
    out = []
    ptr = 0
    for k in range(C):
        o, w = wins[k]
        if ptr < n and dst[ptr] < o:
            return None
        e = ptr
        hi = o + w
        while e < n and e - ptr < P and dst[e] < hi:
            e += 1
        out.append((ptr, e))
        ptr = e
    if ptr != n:
        return None
    return out


def _wrap16(vals):
    """flat int array [n] (n % 128 == 0) -> [128, n//16] int16 in the
    dma_gather layout: flat i at partition i%16, column i//16, replicated
    8x across partition groups."""
    n = len(vals)
    a = np.zeros((16, n // 16), np.int16)
    a[np.arange(n) % 16, np.arange(n) // 16] = vals.astype(np.int16)
    return np.tile(a, (8, 1))


def _assign(core_deg, T):
    """Order a core's items by degree desc; item i -> tile i % T,
    row ((i // T) * 61 + 13) % 128 (scrambled so per-row mass is flat).
    Returns (tile, row) arrays indexed by local item id."""
    n = len(core_deg)
    order = np.argsort(-core_deg, kind="stable")
    tile_ = np.empty(n, np.int32)
    row_ = np.empty(n, np.int32)
    i = np.arange(n)
    tile_[order] = i % T
    row_[order] = ((i // T) * 61 + 13) % P
    return tile_, row_


def _side_tables(dst_core, dst_tile, dst_row, gsrc, T, n_shards, shard_sz,
                 src_limit, NC):
    """Group incidences by (core, tile, shard), sort by dst row, choose a
    uniform chunk count + window schedule per shard, emit idx/loc tables.

    Returns (Cs, wins, per_core list of (idx [T,P,Ct*8] i16,
    loc [T,P,Ct] bf16))."""
    shard = gsrc // shard_sz
    key = ((dst_core.astype(np.int64) * T + dst_tile) * n_shards + shard)
    order = np.lexsort((dst_row, key))
    k_s = key[order]
    d_s = dst_row[order]
    g_s = (gsrc - shard * shard_sz)[order]
    ngrp = NC * T * n_shards
    starts = np.searchsorted(k_s, np.arange(ngrp))
    ends = np.searchsorted(k_s, np.arange(ngrp) + 1)

    # per-shard uniform chunk count with feasibility under the schedule
    Cs = np.zeros(n_shards, np.int64)
    wins_all = []
    packs = {}
    for s in range(n_shards):
        gids = np.arange(ngrp)[np.arange(ngrp) % n_shards == s]
        cnts = ends[gids] - starts[gids]
        C = max(1, int(-(-cnts.max() // P))) if len(cnts) else 1
        base = C
        while True:
            wins = _windows(C)
            ok = True
            pk = {}
            for g in gids:
                dst = d_s[starts[g]:ends[g]]
                r = _pack_group(dst, C, wins)
                if r is None:
                    ok = False
                    break
                pk[g] = r
            if ok:
                break
            C += 1
            if C > base + 6:
                C = base
                wins = [(0, P)] * C
                for g in gids:
                    dst = d_s[starts[g]:ends[g]]
                    pk[g] = _pack_group(dst, C, wins)
                    assert pk[g] is not None
                break
        Cs[s] = C
        wins_all.append(wins)
        packs.update(pk)

    Ct = int(Cs.sum())
    offs = np.concatenate([[0], np.cumsum(Cs)]).astype(int)
    per_core = []
    for c in range(NC):
        idx = np.zeros((T, P, Ct * 8), np.int16)
        loc = np.full((T, P, Ct), PAD_LT, NPBF)
        for t in range(T):
            for s in range(n_shards):
                g = (c * T + t) * n_shards + s
                C = int(Cs[s])
                wins = wins_all[s]
                gv = np.zeros(C * P, np.int64)
                lv = np.full(C * P, PAD_LT, np.float32)
                for k, (a, b) in enumerate(packs[g]):
                    m = b - a
                    sl = slice(k * P, k * P + m)
                    gv[sl] = g_s[starts[g] + a:starts[g] + b]
                    lv[sl] = (d_s[starts[g] + a:starts[g] + b]
                              - wins[k][0])
                idx[t, :, offs[s] * 8:offs[s + 1] * 8] = _wrap16(gv)
                loc[t, :, offs[s]:offs[s + 1]] = (
                    lv.reshape(C, P).T.astype(NPBF))
        per_core.append((idx, loc))
    return Cs, wins_all, per_core


def preprocess(x, hyperedge_index, hyperedge_weight, d):
    ni = np.asarray(hyperedge_index[0]).astype(np.int64)
    ei = np.asarray(hyperedge_index[1]).astype(np.int64)
    w = np.asarray(hyperedge_weight, np.float64)
    NC = d.NC

    edeg = np.bincount(ei, minlength=d.E)
    Dw = np.bincount(ni, weights=w[ei], minlength=d.N)

    e_tile = np.empty(d.E, np.int32)
    e_row = np.empty(d.E, np.int32)
    n_tile = np.empty(d.N, np.int32)
    n_row = np.empty(d.N, np.int32)
    for c in range(NC):
        ge = np.arange(d.ES) * NC + c
        e_tile[ge], e_row[ge] = _assign(edeg[ge], d.T1)
        gn = np.arange(d.NS) * NC + c
        n_tile[gn], n_row[gn] = _assign(
            np.bincount(ni, minlength=d.N)[gn], d.T2)

    e_core = ei % NC
    n_core = ni % NC
    e_grow = (e_core * (d.T1 * P) + e_tile[ei] * P + e_row[ei])
    # e_grow indexed per incidence; also per edge for loc of edges:
    e_growe = (np.arange(d.E) % NC) * (d.T1 * P) + e_tile * P + e_row

    d.C1s, d.W1, side1 = _side_tables(
        e_core, e_tile[ei], e_row[ei], ni, d.T1, d.NSH1, d.SH1, d.N, NC)
    d.C2s, d.W2, side2 = _side_tables(
        n_core, n_tile[ni], n_row[ni], e_growe[ei], d.T2, d.NSH2, d.SH2,
        d.ER, NC)

    binv_g = np.where(edeg > 0, 1.0 / np.maximum(edeg, 1), 0.0)
    dinv_g = np.where(Dw > 0, 1.0 / np.where(Dw > 0, Dw, 1.0), 0.0)

    per_core = []
    for c in range(NC):
        ge = np.arange(d.ES) * NC + c
        bv = np.zeros((P, d.T1), np.float32)
        bv[e_row[ge], e_tile[ge]] = binv_g[ge]
        gn = np.arange(d.NS) * NC + c
        dv = np.zeros((P, d.T2), np.float32)
        dv[n_row[gn], n_tile[gn]] = dinv_g[gn]
        perm = (n_tile[gn] * P + n_row[gn]).astype(np.int64)
        per_core.append(dict(
            g1i=side1[c][0], g1l=side1[c][1],
            g2i=side2[c][0], g2l=side2[c][1],
            binv=bv, dinv=dv,
            _perm=perm,
        ))
    return per_core


def ap3(t_ap, dims_):
    return bass.AP(t_ap.tensor, t_ap.offset, dims_)


def build(d):
    nc = bacc.Bacc("TRN2", target_bir_lowering=False, num_devices=d.NC,
                   num_swdge_queues=4)
    C1t = int(sum(d.C1s))
    C2t = int(sum(d.C2s))
    o1 = np.concatenate([[0], np.cumsum(d.C1s)]).astype(int)
    o2 = np.concatenate([[0], np.cumsum(d.C2s)]).astype(int)
    # one-hot column offsets per shard (widths can differ per shard)
    w1 = [d.W1[s][0][1] for s in range(d.NSH1)]
    w2 = [d.W2[s][0][1] for s in range(d.NSH2)]
    ow1 = np.concatenate([[0], np.cumsum([d.C1s[s] * w1[s]
                                          for s in range(d.NSH1)])]).astype(int)
    ow2 = np.concatenate([[0], np.cumsum([d.C2s[s] * w2[s]
                                          for s in range(d.NSH2)])]).astype(int)
    OHW1 = int(ow1[-1])
    OHW2 = int(ow2[-1])
    ER = d.ER
    YC = P + 1

    x_d = nc.dram_tensor("x", [d.N, P], BF16, kind="ExternalInput")
    w_d = nc.dram_tensor("Wm", [P, P], BF16, kind="ExternalInput")
    gm_d = nc.dram_tensor("gamma", [P, 1], F32, kind="ExternalInput")
    bt_d = nc.dram_tensor("beta", [P, 1], F32, kind="ExternalInput")
    g1i_d = nc.dram_tensor("g1i", [d.T1, P, C1t * 8], I16, kind="ExternalInput")
    g1l_d = nc.dram_tensor("g1l", [d.T1, P, C1t], BF16, kind="ExternalInput")
    g2i_d = nc.dram_tensor("g2i", [d.T2, P, C2t * 8], I16, kind="ExternalInput")
    g2l_d = nc.dram_tensor("g2l", [d.T2, P, C2t], BF16, kind="ExternalInput")
    bv_d = nc.dram_tensor("binv", [P, d.T1], F32, kind="ExternalInput")
    dv_d = nc.dram_tensor("dinv", [P, d.T2], F32, kind="ExternalInput")
    out_d = nc.dram_tensor("out", [d.T2 * P, P], BF16, kind="ExternalOutput")

    iotab_h = nc.inline_tensor(
        np.tile(np.arange(SP2, dtype=NPBF), (P, 1)), name="iota2db")
    ident_h = nc.inline_tensor(np.eye(P, dtype=np.float32), name="ident")

    groups = [list(range(d.NC))]

    with tile.TileContext(nc) as tc:
        with (
            tc.tile_pool(name="const", bufs=1) as cp,
            tc.tile_pool(name="dram", bufs=1, space="DRAM") as dp,
            tc.tile_pool(name="psS", bufs=1, space="PSUM") as psS,
            tc.tile_pool(name="ypool", bufs=1) as yp,
        ):
            IOTB = cp.tile([P, SP2], BF16, name="IOTB")
            nc.sync.dma_start(IOTB[:], iotab_h[:])
            IDN = cp.tile([P, P], F32, name="IDN")
            nc.sync.dma_start(IDN[:], ident_h[:])
            WSB = cp.tile([P, P], BF16, name="WSB")
            nc.sync.dma_start(WSB[:], w_d[:])
            GM = cp.tile([P, 1], F32, name="GM")
            nc.sync.dma_start(GM[:], gm_d[:])
            BT = cp.tile([P, 1], F32, name="BT")
            nc.sync.dma_start(BT[:], bt_d[:])
            BV = cp.tile([P, d.T1], F32, name="BV")
            nc.sync.dma_start(BV[:], bv_d[:])
            DV = cp.tile([P, d.T2], F32, name="DV")
            nc.sync.dma_start(DV[:], dv_d[:])
            ZRO = cp.tile([P, P], BF16, name="ZRO")
            nc.vector.memset(ZRO[:], 0.0)

            e_loc = dp.tile([d.T1 * P, P], BF16, name="e_loc")
            e_full = dp.tile([ER, P], BF16, name="e_full")
            st_in = dp.tile([P, 2], F32, name="st_in")
            st_out = dp.tile([P, 2], F32, name="st_out")

            stats_ps = psS.tile([P, P + 1], F32, name="stats_ps")
            Y = yp.tile([P, d.T2 * YC], BF16, name="Y")

            # ---------------- phase A: node -> edge ----------------
            with (
                tc.tile_pool(name="s1", bufs=3) as s1,
                tc.tile_pool(name="g1", bufs=2) as g1p,
                tc.tile_pool(name="ps1", bufs=2, space="PSUM") as ps1,
                tc.tile_pool(name="pst", bufs=2, space="PSUM") as pst,
            ):
                for t in range(d.T1):
                    it1 = s1.tile([P, C1t * 8], I16, name="it1")
                    nc.sync.dma_start(it1[:], g1i_d[t])
                    lt1 = s1.tile([P, C1t], BF16, name="lt1")
                    nc.sync.dma_start(lt1[:], g1l_d[t])
                    G1 = g1p.tile([P, C1t * P], BF16, name="G1")
                    for s in range(d.NSH1):
                        cs = int(d.C1s[s])
                        base = s * d.SH1
                        sz = min(d.SH1, d.N - base)
                        g_ap = G1[:, o1[s] * P:o1[s + 1] * P]
                        nc.gpsimd.dma_gather(
                            out_ap=ap3(g_ap, [g_ap.ap[0], [P, cs], [1, P]]),
                            in_ap=x_d[base:base + sz, :],
                            idxs_ap=it1[:, o1[s] * 8:o1[s + 1] * 8],
                            num_idxs=cs * P, num_idxs_reg=cs * P,
                            elem_size=P, single_packet=False,
                            queue_num=(t * d.NSH1 + s) % 4)
                    OH = g1p.tile([P, OHW1], BF16, name="OH")
                    for s in range(d.NSH1):
                        cs = int(d.C1s[s])
                        ws = w1[s]
                        oh_ap = OH[:, ow1[s]:ow1[s + 1]]
                        nc.vector.tensor_tensor(
                            out=ap3(oh_ap, [oh_ap.ap[0], [ws, cs], [1, ws]]),
                            in0=lt1[:, o1[s]:o1[s + 1]].to_broadcast(
                                [P, cs, ws]),
                            in1=ap3(IOTB[:], [IOTB[:].ap[0], [0, cs],
                                              [1, ws]]),
                            op=OP.is_equal)
                    peT = ps1.tile([P, P], F32, name="peT")
                    nc.tensor.matmul(peT[:], lhsT=ZRO[:], rhs=IOTB[:, 0:P],
                                     start=True, stop=False,
                                     skip_group_check=True)
                    ci = 0
                    for s in range(d.NSH1):
                        cs = int(d.C1s[s])
                        ws = w1[s]
                        for k in range(cs):
                            ok, wk = d.W1[s][k]
                            nc.tensor.matmul(
                                peT[:, ok:ok + wk],
                                lhsT=G1[:, (o1[s] + k) * P:(o1[s] + k + 1) * P],
                                rhs=OH[:, ow1[s] + k * ws:ow1[s] + (k + 1) * ws],
                                start=False, stop=(ci == C1t - 1),
                                skip_group_check=True)
                            ci += 1
                    agt = s1.tile([P, P], F32, name="agt")
                    nc.vector.tensor_copy(agt[:], peT[:])
                    pt = pst.tile([P, P], F32, name="pt")
                    nc.tensor.transpose(pt[:], agt[:], IDN[:])
                    es = s1.tile([P, P], BF16, name="es")
                    nc.vector.tensor_scalar_mul(
                        out=es[:], in0=pt[:], scalar1=BV[:, t:t + 1])
                    nc.sync.dma_start(e_loc[t * P:(t + 1) * P, :], es[:])

            nc.gpsimd.collective_compute(
                "AllGather", OP.bypass, replica_groups=groups,
                ins=[e_loc[:]], outs=[e_full[:]])

            # ---------------- phase B: edge -> node ----------------
            with (
                tc.tile_pool(name="s2", bufs=3) as s2,
                tc.tile_pool(name="g2", bufs=2) as g2p,
                tc.tile_pool(name="ps2", bufs=2, space="PSUM") as ps2,
                tc.tile_pool(name="psz", bufs=2, space="PSUM") as psz,
            ):
                for t in range(d.T2):
                    it2 = s2.tile([P, C2t * 8], I16, name="it2")
                    nc.sync.dma_start(it2[:], g2i_d[t])
                    lt2 = s2.tile([P, C2t], BF16, name="lt2")
                    nc.sync.dma_start(lt2[:], g2l_d[t])
                    G2 = g2p.tile([P, C2t * P], BF16, name="G2")
                    for s in range(d.NSH2):
                        cs = int(d.C2s[s])
                        base = s * d.SH2
                        sz = min(d.SH2, ER - base)
                        g_ap = G2[:, o2[s] * P:o2[s + 1] * P]
                        nc.gpsimd.dma_gather(
                            out_ap=ap3(g_ap, [g_ap.ap[0], [P, cs], [1, P]]),
                            in_ap=e_full[base:base + sz, :],
                            idxs_ap=it2[:, o2[s] * 8:o2[s + 1] * 8],
                            num_idxs=cs * P, num_idxs_reg=cs * P,
                            elem_size=P, single_packet=False,
                            queue_num=(t * d.NSH2 + s) % 4)
                    OH2 = g2p.tile([P, OHW2], BF16, name="OH2")
                    for s in range(d.NSH2):
                        cs = int(d.C2s[s])
                        ws = w2[s]
                        oh_ap = OH2[:, ow2[s]:ow2[s + 1]]
                        nc.vector.tensor_tensor(
                            out=ap3(oh_ap, [oh_ap.ap[0], [ws, cs], [1, ws]]),
                            in0=lt2[:, o2[s]:o2[s + 1]].to_broadcast(
                                [P, cs, ws]),
                            in1=ap3(IOTB[:], [IOTB[:].ap[0], [0, cs],
                                              [1, ws]]),
                            op=OP.is_equal)
                    snT = ps2.tile([P, P], F32, name="snT")
                    nc.tensor.matmul(snT[:], lhsT=ZRO[:], rhs=IOTB[:, 0:P],
                                     start=True, stop=False,
                                     skip_group_check=True)
                    ci = 0
                    for s in range(d.NSH2):
                        cs = int(d.C2s[s])
                        ws = w2[s]
                        for k in range(cs):
                            ok, wk = d.W2[s][k]
                            nc.tensor.matmul(
                                snT[:, ok:ok + wk],
                                lhsT=G2[:, (o2[s] + k) * P:(o2[s] + k + 1) * P],
                                rhs=OH2[:, ow2[s] + k * ws:ow2[s] + (k + 1) * ws],
                                start=False, stop=(ci == C2t - 1),
                                skip_group_check=True)
                            ci += 1
                    agT = s2.tile([P, P], BF16, name="agT")
                    nc.vector.tensor_copy(agT[:], snT[:])
                    z = psz.tile([P, P], F32, name="z")
                    nc.tensor.matmul(z[:], lhsT=agT[:], rhs=WSB[:],
                                     start=True, stop=True)
                    yc = t * YC
                    nc.vector.tensor_scalar_mul(
                        out=Y[:, yc:yc + P], in0=z[:],
                        scalar1=DV[:, t:t + 1])
                    nc.vector.memset(Y[:, yc + P:yc + P + 1], 1.0)
                    nc.tensor.matmul(stats_ps[:], lhsT=Y[:, yc:yc + P],
                                     rhs=Y[:, yc:yc + P + 1],
                                     start=(t == 0), stop=(t == d.T2 - 1))

            # ---------------- phase C: BN stats ----------------
            with (
                tc.tile_pool(name="s3", bufs=1) as s3,
                tc.tile_pool(name="ps3", bufs=2, space="PSUM") as ps3,
            ):
                sts = s3.tile([P, P + 1], F32, name="sts")
                nc.vector.tensor_copy(sts[:], stats_ps[:])
                dg = s3.tile([P, P], F32, name="dg")
                nc.vector.tensor_tensor(out=dg[:], in0=sts[:, 0:P],
                                        in1=IDN[:], op=OP.mult)
                st2 = s3.tile([P, 2], F32, name="st2")
                nc.vector.tensor_reduce(out=st2[:, 1:2], in_=dg[:],
                                        axis=mybir.AxisListType.X, op=OP.add)
                nc.vector.tensor_copy(st2[:, 0:1], sts[:, P:P + 1])
                nc.sync.dma_start(st_in[:], st2[:])
                nc.gpsimd.collective_compute(
                    "AllReduce", OP.add, replica_groups=groups,
                    ins=[st_in[:]], outs=[st_out[:]])
                gst = s3.tile([P, 2], F32, name="gst")
                nc.sync.dma_start(gst[:], st_out[:])
                mean = s3.tile([P, 1], F32, name="mean")
                nc.vector.tensor_scalar_mul(out=mean[:], in0=gst[:, 0:1],
                                            scalar1=1.0 / d.N)
                var = s3.tile([P, 1], F32, name="var")
                nc.vector.tensor_scalar_mul(out=var[:], in0=gst[:, 1:2],
                                            scalar1=1.0 / d.N)
                m2 = s3.tile([P, 1], F32, name="m2")
                nc.vector.tensor_tensor(out=m2[:], in0=mean[:], in1=mean[:],
                                        op=OP.mult)
                nc.vector.tensor_tensor(out=var[:], in0=var[:], in1=m2[:],
                                        op=OP.subtract)
                epsl = s3.tile([P, 1], F32, name="epsl")
                nc.vector.memset(epsl[:], d.BN_EPS)
                sd = s3.tile([P, 1], F32, name="sd")
                nc.scalar.activation(out=sd[:], in_=var[:], func=AF.Sqrt,
                                     bias=epsl[:])
                nc.vector.reciprocal(sd[:], sd[:])
                scl = s3.tile([P, 1], F32, name="scl")
                nc.vector.tensor_tensor(out=scl[:], in0=GM[:], in1=sd[:],
                                        op=OP.mult)
                sft = s3.tile([P, 1], F32, name="sft")
                nc.vector.tensor_tensor(out=sft[:], in0=mean[:], in1=scl[:],
                                        op=OP.mult)
                nc.vector.tensor_tensor(out=sft[:], in0=BT[:], in1=sft[:],
                                        op=OP.subtract)
                pb = ps3.tile([P, P], F32, name="pb")
                nc.tensor.transpose(pb[:], scl[:].to_broadcast([P, P]),
                                    IDN[:])
                SCL = s3.tile([P, P], BF16, name="SCL")
                nc.vector.tensor_copy(SCL[:], pb[:])
                pb2 = ps3.tile([P, P], F32, name="pb2")
                nc.tensor.transpose(pb2[:], sft[:].to_broadcast([P, P]),
                                    IDN[:])
                SFT = s3.tile([P, P], BF16, name="SFT")
                nc.vector.tensor_copy(SFT[:], pb2[:])

                # ---------------- phase D: finalize ----------------
                with tc.tile_pool(name="s4", bufs=3) as s4:
                    for t in range(d.T2):
                        yc = t * YC
                        yt = s4.tile([P, P], BF16, name="yt")
                        nc.vector.tensor_tensor(out=yt[:],
                                                in0=Y[:, yc:yc + P],
                                                in1=SCL[:], op=OP.mult)
                        nc.vector.tensor_tensor(out=yt[:], in0=yt[:],
                                                in1=SFT[:], op=OP.add)
                        ot = s4.tile([P, P], BF16, name="ot")
                        nc.scalar.activation(out=ot[:], in_=yt[:],
                                             func=AF.Silu)
                        nc.sync.dma_start(out_d[t * P:(t + 1) * P, :], ot[:])
    nc.compile()
    return nc


def _run(d, x, W, gamma, beta, per_core, trace=False, **rkw):
    nc = build(d)
    xbf = np.ascontiguousarray(x.astype(NPBF))
    Wbf = np.ascontiguousarray(W.astype(NPBF))
    in_maps = []
    for c in range(d.NC):
        m = {k: v for k, v in per_core[c].items() if not k.startswith("_")}
        m["x"] = xbf
        m["Wm"] = Wbf
        m["gamma"] = gamma.reshape(P, 1)
        m["beta"] = beta.reshape(P, 1)
        in_maps.append(m)
    res = run_bass_kernel_spmd(nc, in_maps, core_ids=list(range(d.NC)),
                               trace=trace, **rkw)
    out = np.empty((d.N, P), np.float32)
    for c in range(d.NC):
        perm = per_core[c]["_perm"]
        oc = np.asarray(res.results[c]["out"]).astype(np.float32)
        out[c::d.NC] = oc[perm]
    return out, res


def kernel(x, hyperedge_index, hyperedge_weight, W, b, gamma, beta):
    x = np.ascontiguousarray(np.asarray(x, np.float32))
    W = np.ascontiguousarray(np.asarray(W, np.float32))
    gamma = np.asarray(gamma, np.float32)
    beta = np.asarray(beta, np.float32)
    d = Dims(N=x.shape[0], E=np.asarray(hyperedge_weight).shape[0],
             NNZ=np.asarray(hyperedge_index).shape[1], n_cores=8)
    per_core = preprocess(x, hyperedge_index, hyperedge_weight, d)
    out, _ = _run(d, x, W, gamma, beta, per_core)
    return out
